# revision 19
# baseline (speedup 1.0000x reference)
"""Two-layer single-head GAT (GATConv x2) on 8 trn2 NeuronCores.

Strategy: 1D node partition across 8 cores by destination node; edges live
with their destination owner, so edge-softmax and the scatter-aggregate stay
local. Weights replicated. Both layers share one graph plan.

v2 design: NO dma_gather (the old SWDGE descriptor generation cost ~8ns per
gathered row and was 90% of the runtime). Instead the host lays out hT with
ONE COLUMN PER EDGE SLOT (slot-major: column (j*128+p) = source feature of
slot j for destination-partition p of its tile), so the Stage A matmul
computes every edge's table row directly in the order Stage B consumes it.
The "table" never exists: matmul -> PSUM -> SBUF copy -> edge-softmax math.

Per layer, per core:
  Pre: ad = h_own @ (W @ a_dst) per destination (one matmul column per tile).
  Per gather-group (consecutive tiles, sum K <= GB slot-columns):
    xt   = DMA of hT slot columns (contiguous, HWDGE, no descriptors/idx)
    psum = xt_chunk.T @ wse[:, 0:65]   (cols 0:64 = W*a_src -> T row,
                                        col 64 = W@a_src   -> as = alpha_src)
    tg   = PSUM copied to SBUF on the Scalar engine  [128, KG, 65]
    per tile: z = as + ad ; s = LeakyReLU(z) ; p = Exp(s), den accumulated
    rd = 1/den (batched per group)
    U  = sum_k p_k T_k (DVE mult + transposed reduce)
    out = U * (1/a_src) * rd + b, staged per group, one DMA per group
  Padded slots get a host-built pad column = -1e30 * v / (v.v), v = W@a_src,
  so as = -1e30 exactly (all products negative) => p == 0.
"""

import sys

sys.path.insert(0, "/opt/trn_rl_repo")

import numpy as np

N = 50000
E = 800000
IN = 128
OUT = 64
C = 8                       # cores
NSH = N // C                # 6250 dsts per core
NTILES = (NSH + 127) // 128  # 49
NSHP = NTILES * 128         # 6272 padded dsts per core
NEG_SLOPE = 0.2
GB = 64                     # max slot-columns per gather group
SCH = 7                     # matmul chunks per PSUM bank (7*65 = 455 <= 512)
PAD_AS = -60000.0           # fp16-safe; LeakyReLU then exp underflows to 0


def _build_plan(edge_index):
    """Host-side graph preprocessing shared by both layers."""
    src = np.concatenate([np.asarray(edge_index[0], dtype=np.int64), np.arange(N)])
    dst = np.concatenate([np.asarray(edge_index[1], dtype=np.int64), np.arange(N)])

    core_of = dst // NSH
    orders = []
    pos_of = np.empty(N, dtype=np.int64)
    for c in range(C):
        d0 = c * NSH
        deg_c = np.bincount(dst[core_of == c] - d0, minlength=NSH)
        order = np.argsort(-deg_c, kind="stable")
        pos_of[d0 + order] = np.arange(NSH)
        orders.append(np.concatenate([order + d0, np.full(NSHP - NSH, -1, np.int64)]))

    epos = pos_of[dst]

    # per-tile K = max degree (over cores)
    K = np.zeros(NTILES, np.int64)
    for c in range(C):
        deg_p = np.bincount(epos[core_of == c], minlength=NSHP)
        K = np.maximum(K, deg_p.reshape(NTILES, 128).max(1))

    # per-edge slot assignment: rank within (core, pos)
    okey = np.lexsort((epos, core_of))
    sc, pc, srt = core_of[okey], epos[okey], src[okey]
    gid = sc * NSHP + pc
    first = np.r_[True, gid[1:] != gid[:-1]]
    idx_lin = np.arange(len(gid))
    start = np.maximum.accumulate(np.where(first, idx_lin, 0))
    rank = idx_lin - start
    assert (rank < K[(pc // 128)]).all()

    tile_off = np.concatenate([[0], np.cumsum(K)[:-1]])
    SLOTS = int(K.sum())

    # per-core slot-major source-column map; pad columns point at index N
    col = (tile_off[pc // 128] + rank) * 128 + (pc % 128)
    colsrc = np.full((C, SLOTS * 128), N, np.int64)
    colsrc[sc, col] = srt

    # gather groups: consecutive tiles, sum K <= GB
    groups = []
    cur, acc = [], 0
    for t in range(NTILES):
        k = int(K[t])
        if cur and acc + k > GB:
            groups.append((cur, int(tile_off[cur[0]]), acc))
            cur, acc = [], 0
        cur.append(t)
        acc += k
    if cur:
        groups.append((cur, int(tile_off[cur[0]]), acc))

    return orders, colsrc, K, tile_off, SLOTS, groups


def _build_launch(kdim, K, tile_off, SLOTS, groups):
    """One SPMD launch: matmul table rows straight into per-group SBUF tiles,
    then edge-softmax + weighted aggregate per destination tile."""
    import concourse.bacc as bacc
    import concourse.mybir as mybir
    from concourse.tile import TileContext

    f32 = mybir.dt.float32
    bf16 = mybir.dt.bfloat16
    fp16 = mybir.dt.float16
    GBMAX = max(kg for (_, _, kg) in groups)
    MAXT = max(len(tl) for (tl, _, _) in groups)
    KMAX = int(K.max())

    nc = bacc.Bacc(None, target_bir_lowering=False, debug=True)
    hT = nc.declare_dram_parameter("hT", [kdim, SLOTS * 128], bf16, isOutput=False)
    hoT = nc.declare_dram_parameter("hoT", [kdim, NSHP], bf16, isOutput=False)
    wse = nc.declare_dram_parameter("wse", [kdim, 66], bf16, isOutput=False)
    rb = nc.declare_dram_parameter("rb", [128, 128], f32, isOutput=False)
    outp = nc.declare_dram_parameter("outp", [NSHP, 64], f32, isOutput=True)

    with TileContext(nc) as tc:
        with (
            tc.tile_pool(name="const", bufs=1) as cpool,
            tc.tile_pool(name="xin", bufs=3) as xin,
            tc.tile_pool(name="psA", bufs=4, space="PSUM") as psA,
            tc.tile_pool(name="psB", bufs=2, space="PSUM") as psB,
            tc.tile_pool(name="tg", bufs=3) as tgp,
            tc.tile_pool(name="pt", bufs=2) as ptp,
            tc.tile_pool(name="sm", bufs=3) as sm,
        ):
            wse_sb = cpool.tile([kdim, 66], bf16)
            nc.sync.dma_start(out=wse_sb[:, :], in_=wse[:, :])
            rb_sb = cpool.tile([128, 128], f32)
            nc.sync.dma_start(out=rb_sb[:, :], in_=rb[:, :])
            ho_sb = cpool.tile([kdim, NSHP], bf16)
            nc.sync.dma_start(out=ho_sb[:, :], in_=hoT[:, :])
            ad_sb = cpool.tile([128, NTILES], f32)

            # per-destination ad = h_own . (W @ a_dst)
            for t in range(NTILES):
                ps2 = psB.tile([128, 1], f32, tag="ps2")
                nc.tensor.matmul(ps2[:, :], ho_sb[:, 128 * t:128 * (t + 1)],
                                 wse_sb[:, 65:66], start=True, stop=True)
                nc.scalar.copy(ad_sb[:, t:t + 1], ps2[:, :])

            for (tiles, j0, KG) in groups:
                xt = xin.tile([kdim, GBMAX * 128], bf16, tag="xt")
                nc.sync.dma_start(out=xt[:, 0:KG * 128],
                                  in_=hT[:, j0 * 128:(j0 + KG) * 128])
                tg = tgp.tile([128, GBMAX, 65], fp16, tag="tg")
                for c0 in range(0, KG, SCH):
                    nch = min(SCH, KG - c0)
                    ps = psA.tile([128, SCH * 65], f32, tag="ps")
                    for j in range(nch):
                        nc.tensor.matmul(ps[:, 65 * j:65 * (j + 1)],
                                         xt[:, 128 * (c0 + j):128 * (c0 + j + 1)],
                                         wse_sb[:, 0:65], start=True, stop=True)
                    nc.scalar.copy(tg[:, c0:c0 + nch, :],
                                   ps[:, 0:65 * nch]
                                   .rearrange("p (c f) -> p c f", f=65))

                ng = len(tiles)
                den_g = sm.tile([128, MAXT], f32, tag="den")
                p_list = []
                for i, t in enumerate(tiles):
                    toff = int(tile_off[t]) - j0
                    k = int(K[t])
                    z_t = sm.tile([128, k], f32, tag=f"z{i}")
                    nc.vector.tensor_tensor(z_t[:, :],
                                            tg[:, toff:toff + k, 64:65].squeeze(2),
                                            ad_sb[:, t:t + 1]
                                            .broadcast_to([128, k]),
                                            mybir.AluOpType.add)
                    s_t = sm.tile([128, k], f32, tag=f"s{i}")
                    nc.vector.scalar_tensor_tensor(s_t[:, :], z_t[:, :],
                                                   NEG_SLOPE, z_t[:, :],
                                                   mybir.AluOpType.mult,
                                                   mybir.AluOpType.max)
                    p_t = sm.tile([128, k], f32, tag=f"p{i}")
                    nc.scalar.activation(p_t[:, :], s_t[:, :],
                                         mybir.ActivationFunctionType.Exp,
                                         accum_out=den_g[:, i:i + 1])
                    p_list.append((p_t, t, toff, k, i))
                rd_g = sm.tile([128, MAXT], f32, tag="rd")
                nc.vector.reciprocal(rd_g[:, 0:ng], den_g[:, 0:ng])

                og = sm.tile([128, MAXT * 64], f32, tag="og")
                for (p_t, t, toff, k, i) in p_list:
                    al_t = sm.tile([128, k], fp16, tag=f"al{i}")
                    nc.vector.tensor_tensor(al_t[:, :], p_t[:, :],
                                            rd_g[:, i:i + 1]
                                            .broadcast_to([128, k]),
                                            mybir.AluOpType.mult)
                    pt = ptp.tile([128, KMAX, 64], fp16, tag="pt")
                    a_b = al_t[:, :].unsqueeze(2).broadcast_to([128, k, 64])
                    nc.vector.tensor_tensor(pt[:, 0:k, :],
                                            tg[:, toff:toff + k, 0:64], a_b,
                                            mybir.AluOpType.mult)
                    u = sm.tile([128, 64], f32, tag=f"u{i}")
                    nc.vector.tensor_reduce(u[:, :],
                                            pt[:, 0:k, :].transpose([0, 2, 1]),
                                            mybir.AxisListType.X,
                                            mybir.AluOpType.add)
                    o1 = sm.tile([128, 64], f32, tag=f"o1{i}")
                    nc.vector.tensor_tensor(o1[:, :], u[:, :], rb_sb[:, 0:64],
                                            mybir.AluOpType.mult)
                    nc.vector.tensor_tensor(og[:, 64 * i:64 * (i + 1)],
                                            o1[:, :], rb_sb[:, 64:128],
                                            mybir.AluOpType.add)
                t0 = tiles[0]
                nc.sync.dma_start(
                    out=outp[128 * t0:128 * (t0 + ng), :]
                    .rearrange("(c p) f -> p c f", p=128),
                    in_=og[:, 0:ng * 64].rearrange("p (c f) -> p c f", f=64))

    nc.compile()
    return nc


LAST = {}


def _pad_col(W, a_src):
    """Column c with (W*a_src).T @ c == (PAD_AS/64) * ones(64), so every
    feature of the pad slot's T row is ~-940 (fp16-finite) and their sum --
    the as column -- is PAD_AS exactly."""
    Wsc = np.asarray(W, np.float64) * np.asarray(a_src, np.float64)[None, :]
    rhs = np.full(Wsc.shape[1], PAD_AS / Wsc.shape[1])
    c, *_ = np.linalg.lstsq(Wsc.T, rhs, rcond=None)
    return c


def kernel(x, edge_index, W1, a_src1, a_dst1, b1, W2, a_src2, a_dst2, b2):
    from concourse.bass_utils import run_bass_kernel_spmd
    import ml_dtypes

    bf = np.dtype(ml_dtypes.bfloat16)

    x = np.asarray(x, np.float32)
    edge_index = np.asarray(edge_index)
    W1 = np.asarray(W1, np.float32); a_src1 = np.asarray(a_src1, np.float32)
    a_dst1 = np.asarray(a_dst1, np.float32); b1 = np.asarray(b1, np.float32)
    W2 = np.asarray(W2, np.float32); a_src2 = np.asarray(a_src2, np.float32)
    a_dst2 = np.asarray(a_dst2, np.float32); b2 = np.asarray(b2, np.float32)

    orders, colsrc, K, tile_off, SLOTS, groups = _build_plan(edge_index)

    nc1 = _build_launch(IN, K, tile_off, SLOTS, groups)
    nc2 = _build_launch(OUT, K, tile_off, SLOTS, groups)

    def guard(a):
        return np.where(a == 0, np.float32(1e-30), a)

    def make_wse(W, a_src, a_dst):
        return np.concatenate([W * a_src[None, :], (W @ a_src)[:, None],
                               (W @ a_dst)[:, None]], 1).astype(bf)

    wse1 = make_wse(W1, a_src1, a_dst1)
    wse2 = make_wse(W2, a_src2, a_dst2)
    rb1 = np.concatenate([np.tile(1.0 / guard(a_src1), (128, 1)),
                          np.tile(b1, (128, 1))], 1).astype(np.float32)
    rb2 = np.concatenate([np.tile(1.0 / guard(a_src2), (128, 1)),
                          np.tile(b2, (128, 1))], 1).astype(np.float32)

    # layer 1 inputs
    xTpad = np.concatenate([x.T, _pad_col(W1, a_src1)[:, None]], 1).astype(bf)
    in_maps1 = []
    for c in range(C):
        own = orders[c]
        hoT = np.zeros((IN, NSHP), bf)
        real = own >= 0
        hoT[:, real] = xTpad[:, own[real]]
        in_maps1.append({"hT": np.ascontiguousarray(xTpad[:, colsrc[c]]),
                         "hoT": hoT, "wse": wse1, "rb": rb1})

    res1 = run_bass_kernel_spmd(nc1, in_maps1, core_ids=list(range(C)))
    LAST["res1"] = res1

    # h2 per node from pi-order shards
    h2 = np.zeros((N, OUT), np.float32)
    for c in range(C):
        sh = np.asarray(res1.results[c]["outp"])
        own = orders[c]
        real = own >= 0
        h2[own[real]] = sh[real]

    h2Tpad = np.concatenate([h2.T, _pad_col(W2, a_src2)[:, None]], 1).astype(bf)
    in_maps2 = []
    for c in range(C):
        own = orders[c]
        hoT2 = np.zeros((OUT, NSHP), bf)
        real = own >= 0
        hoT2[:, real] = h2Tpad[:, own[real]]
        in_maps2.append({"hT": np.ascontiguousarray(h2Tpad[:, colsrc[c]]),
                         "hoT": hoT2, "wse": wse2, "rb": rb2})

    res2 = run_bass_kernel_spmd(nc2, in_maps2, core_ids=list(range(C)))
    LAST["res2"] = res2

    out = np.empty((N, OUT), np.float32)
    for c in range(C):
        sh = np.asarray(res2.results[c]["outp"])
        own = orders[c]
        real = own >= 0
        out[own[real]] = sh[real]
    return out
